# revision 30
# baseline (speedup 1.0000x reference)
"""Bass/Trainium2 kernel for nn_Expert_WNO2d (8-expert gated WaveConv2d mixture).

Math: the reference is linear in x. Every expert passes the fine Haar detail
levels (1..3) through unchanged and only channel-mixes the coarsest (level-4)
approximation + detail coefficients. With gate slots s weighting experts
PERM = (0,1,2,3,4,5,4,5), the output collapses to

    y[b] = G[b] * x[b] + rep8( adj[b] )                      (rep8 = 8x8 block broadcast)
    adj[b] = 0.125 * idwt4( sum_e geff[b,e] * (W_e . c4[b]) ) - (G[b]/64) * s8[b]

where s8 = 8x8 block sums of x, c4 = level-4 Haar coefficients (from s8),
G[b] = sum_s lambda[b,s], geff[b,e] = gate mass routed to expert e.

Sharding: data-parallel over batch B=32 across 8 cores (4 samples/core).
Memory-bound: x in bf16, y out bf16, expert weights fp8 e4m3 (scaled 2^16
against fp8 subnormals; descaled on the tiny adj tensor). Block sums run as
two bf16 tensor-tensor folds (DVE 2x mode) + a small multi-axis reduce.
Gate factors ship pre-expanded (bf16, packed) so the coefficient scaling
also hits DVE 2x. Mixing matmuls use 128-col fp8 stationaries (two modes
packed, par-diagonal PSUM quadrants) with fast-weight-load, pipelined per
band against the weight DMA stream. The final y = G*x + rep8(adj) pass is
split: Vector (fused STT, rt0), GpSimd (premultiplied G*x early + adj add
late), TensorE (diag(G) @ x + I @ adj_bcast into PSUM, ACT copies to bf16).
Input DMA owns the Sync queue; y stores follow there.
"""

import numpy as np

import concourse.bacc as bacc
import concourse.mybir as mybir
import concourse.tile as tile

N_CORES = 8
B, C, S = 32, 64, 64
BL = B // N_CORES          # samples per core = 4
f32 = mybir.dt.float32
bf16 = mybir.dt.bfloat16
f8 = mybir.dt.float8e4
ALU = mybir.AluOpType
WSCALE = 2.0 ** 16         # fp8 weight scale (weights ~1e-5 underflow fp8 otherwise)


def _build_nc():
    nc = bacc.Bacc()
    xw = nc.declare_dram_parameter("xw", [2, 128, 4096], bf16, isOutput=False)
    wt = nc.declare_dram_parameter("wt", [4, 128, 3072], f8, isOutput=False)
    gt = nc.declare_dram_parameter("gt", [128, 16], f32, isOutput=False)
    dg = nc.declare_dram_parameter("dg", [2, 128, 128], bf16, isOutput=False)
    iw = nc.declare_dram_parameter("iw", [128, 128], bf16, isOutput=False)
    yw = nc.declare_dram_parameter("yw", [2, 128, 4096], bf16, isOutput=True)

    with tile.TileContext(nc) as tc:
        with (
            tc.tile_pool(name="xp", bufs=4) as xp,
            tc.tile_pool(name="yp", bufs=8) as yp,
            tc.tile_pool(name="wp", bufs=4) as wp,
            tc.tile_pool(name="fp", bufs=2) as fpp,
            tc.tile_pool(name="sp", bufs=2) as sp,
            tc.tile_pool(name="ps", bufs=1, space="PSUM") as psp,
            tc.tile_pool(name="py", bufs=3, space="PSUM") as psy,
        ):
            # ---- input DMA on Sync: x chunks (a=3072, b=1024 cols), gt, ge, w
            gt_s = sp.tile([128, 16], f32, tag="gt", name="gt")
            dg_s = sp.tile([128, 256], bf16, tag="dg", name="dg")
            nc.scalar.dma_start(out=dg_s[:, 0:128], in_=dg[0, :, :])
            nc.scalar.dma_start(out=dg_s[:, 128:256], in_=dg[1, :, :])
            iw_s = sp.tile([128, 128], bf16, tag="iw", name="iw")
            nc.scalar.dma_start(out=iw_s[:, :], in_=iw[:, :])

            CHW = (3072, 1024)                       # chunk widths per rt
            xc = []                                  # xc[rt*2+h]
            for rt in range(2):
                for h in range(2):
                    wdt = CHW[h]
                    xt = xp.tile([128, wdt], bf16, tag=f"xs{h}", name=f"x{rt}{h}")
                    nc.sync.dma_start(out=xt[:, :], in_=xw[rt, :, 3072 * h:3072 * h + wdt],
                                      single_packet=True)
                    xc.append(xt)

            nc.sync.dma_start(out=gt_s[:, :], in_=gt[:, :])

            wt_b = []
            for band in range(4):
                w = wp.tile([128, 3072], f8, tag="wt", name=f"w{band}")
                nc.sync.dma_start(out=w[:, :], in_=wt[band, :, :], single_packet=True)
                wt_b.append(w)

            # ---- 8x8 block sums: two bf16 w-folds (DVE 2x) + small XY reduce
            s8 = [sp.tile([128, 64], f32, tag="s8", name=f"s8{rt}") for rt in range(2)]
            with nc.allow_low_precision("bf16 partial block sums; feed a small "
                                        "additive term, within 2e-2 gate"):
                for rt in range(2):
                    for h in range(2):
                        u = (6, 2)[h]
                        src = xc[rt * 2 + h]
                        n1 = u * 8 * 8 * 4
                        feng = nc.vector if h == 0 else nc.gpsimd
                        f1 = fpp.tile([128, n1], bf16, tag=f"f1{h}", name=f"f1{rt}{h}")
                        v4 = lambda t, w: t[:, :].rearrange(
                            "p (r w) -> p r w", w=w)
                        feng.tensor_add(
                            v4(f1, 4),
                            src[:, :].rearrange("p (r w) -> p r w", w=8)[:, :, 0:4],
                            src[:, :].rearrange("p (r w) -> p r w", w=8)[:, :, 4:8])
                        f2 = fpp.tile([128, n1 // 2], bf16, tag=f"f2{h}", name=f"f2{rt}{h}")
                        feng.tensor_add(
                            v4(f2, 2), v4(f1, 4)[:, :, 0:2], v4(f1, 4)[:, :, 2:4])
                        nc.vector.tensor_reduce(
                            out=s8[rt][:, 48 * h:48 * h + 8 * u]
                                .rearrange("p (u v) -> p u v", u=u),
                            in_=f2[:, :].rearrange(
                                "p (u dh v w) -> p u v dh w", u=u, dh=8, v=8, w=2),
                            axis=mybir.AxisListType.XY, op=ALU.add,
                        )

            # ---- level-4 Haar analysis + gate-scaled bf16 coefficients
            # cf col = band*16 + m (band order ll,lh,hl,hh; m = x*4+y = pr*2+par)
            # cc[el*64+i, b*192 + ch*64 + band*16 + m]; ge carries geff*0.0625
            cc = sp.tile([128, 768], f8, tag="cc", name="cc")
            with nc.allow_low_precision("bf16 coefficient chain; feeds a small "
                                        "additive term, within 2e-2 gate"):
                for rt in range(2):
                    ev = s8[rt][:, :].rearrange("p (x i y j) -> p i j x y",
                                                x=4, i=2, y=4, j=2)[:, :, 0]
                    od = s8[rt][:, :].rearrange("p (x i y j) -> p i j x y",
                                                x=4, i=2, y=4, j=2)[:, :, 1]
                    tt = sp.tile([128, 64], bf16, tag="tt", name=f"tt{rt}")
                    t2v = lambda o: tt[:, 32 * o:32 * (o + 1)].rearrange(
                        "p (g x y) -> p g x y", g=2, x=4, y=4)
                    nc.vector.tensor_add(t2v(0), ev, od)
                    nc.vector.tensor_sub(t2v(1), ev, od)
                    cf = sp.tile([128, 64], bf16, tag="coef", name=f"cf{rt}")
                    pick = lambda t, o: t[:, :].rearrange(
                        "p (g h m) -> p h g m", g=2, h=2, m=16)[:, o]
                    nc.vector.tensor_add(pick(cf, 0), pick(tt, 0), pick(tt, 1))  # ll, hl
                    nc.vector.tensor_sub(pick(cf, 1), pick(tt, 0), pick(tt, 1))  # lh, hh

                    for bh in range(2):
                        b = rt * 2 + bh
                        for el in range(2):
                            ceng = nc.vector if el == 0 else nc.gpsimd
                            ceng.tensor_tensor(
                                out=cc[el * 64:(el + 1) * 64, b::4].rearrange(
                                    "p (ch band m) -> p ch band m", ch=3, band=4, m=16),
                                in0=cf[bh * 64:(bh + 1) * 64, :].rearrange(
                                    "p (o bm) -> p o bm", o=1).broadcast_to([64, 3, 64])
                                    .rearrange("p c (band m) -> p c band m", band=4),
                                in1=gt_s[bh * 64:(bh + 1) * 64,
                                         8 * rt + 1 + el:8 * rt + 6 + el:2]
                                    .rearrange("p (ch o u) -> p ch o u", ch=3, o=1, u=1)
                                    .broadcast_to([64, 3, 4, 16]),
                                op=ALU.mult,
                            )

            # ---- per-mode channel mixing: 128-col fp8 stationaries (mode pairs),
            # rhs cols (b, par); only par-diagonal PSUM quadrants are valid.
            # Band-ordered so band k's matmuls start as soon as w[k] lands.
            warm = psp.tile([128, 512], f32, tag="warm", name="warm")
            for wi in range(8):
                wsrc = xc[2] if wi < 6 else xc[3]
                woff = 512 * wi if wi < 6 else 512 * (wi - 6)
                nc.tensor.matmul(
                    out=warm[:, :], lhsT=dg_s[:, 0:128],
                    rhs=wsrc[:, woff:woff + 512], start=True, stop=True,
                )
            pbt = psp.tile([128, 256], f32, tag="pb", name="pb")
            pb = [pbt[:, 64 * i:64 * (i + 1)] for i in range(4)]
            for band in range(4):
                for pr in range(8):
                    base = ch0 = band * 64 + pr * 8
                    for ch in range(3):
                        nc.tensor.matmul(
                            out=pb[band][:, pr * 8:(pr + 1) * 8],
                            lhsT=wt_b[band][:, (pr * 3 + ch) * 128:(pr * 3 + ch + 1) * 128],
                            rhs=cc[:, ch * 256 + base:ch * 256 + base + 8],
                            start=(ch == 0), stop=(ch == 2),
                        )

            # ---- level-4 Haar synthesis from PSUM quadrants (rt1 first: it
            # gates the PE/GpSimd pieces)
            # SD[:, di*64 + pr*8 + b*2 + par]: di=0 -> ll+lh, di=1 -> ll-lh
            SD = sp.tile([128, 128], f32, tag="SD", name="SD")
            TU = sp.tile([128, 128], f32, tag="TU", name="TU")
            sb1 = sp.tile([128, 64], f32, tag="sb1", name="sb1")
            sb3 = sp.tile([128, 64], f32, tag="sb3", name="sb3")
            nc.vector.tensor_copy(sb1[:, :], pb[1][:, :])
            nc.vector.tensor_copy(sb3[:, :], pb[3][:, :])
            nc.vector.tensor_add(SD[:, 0:64], pb[0][:, :], sb1[:, :])
            nc.vector.tensor_sub(SD[:, 64:128], pb[0][:, :], sb1[:, :])
            nc.vector.tensor_add(TU[:, 0:64], pb[2][:, :], sb3[:, :])
            nc.vector.tensor_sub(TU[:, 64:128], pb[2][:, :], sb3[:, :])

            # at[bh*64+o, x*16 + di*8 + yy*4 + par*2 + dj] = spatial 8x8 adj block
            adjH = [None, None]
            for rt in (1, 0):
                at = sp.tile([128, 64], f32, tag="at", name=f"at{rt}")
                for bh in range(2):
                    b = rt * 2 + bh
                    ov = at[bh * 64:(bh + 1) * 64, :].rearrange(
                        "p (x di yy par dj) -> p x di yy par dj",
                        x=4, di=2, yy=2, par=2, dj=2)
                    sv = lambda t, par: t[par * 64:(par + 1) * 64, :].rearrange(
                        "p (di x yy pq bb) -> p x di yy pq bb",
                        di=2, x=4, yy=2, pq=2, bb=4)[:, :, :, :, par, b]
                    for par in range(2):
                        nc.vector.tensor_add(ov[:, :, :, :, par, 0], sv(SD, par), sv(TU, par))
                        nc.vector.tensor_sub(ov[:, :, :, :, par, 1], sv(SD, par), sv(TU, par))
                # adjF = at + (-G/64 * WSCALE) * s8  (still scaled by WSCALE)
                af = sp.tile([128, 64], f32, tag="adjF", name=f"af{rt}")
                nc.vector.scalar_tensor_tensor(
                    out=af[:, :], in0=s8[rt][:, :], scalar=gt_s[:, 8 * rt + 7:8 * rt + 8],
                    in1=at[:, :], op0=ALU.mult, op1=ALU.add,
                )
                # expand over h-rep and descale: adj_h[p, u*64+dh*8+v] = adjF/WSCALE
                ah = sp.tile([128, 512], bf16, tag="adjh", name=f"ah{rt}")
                nc.vector.tensor_scalar(
                    out=ah[:, :].rearrange("p (u dh v) -> p u dh v", u=8, dh=8, v=8),
                    in0=af[:, :].rearrange("p (u o v) -> p u o v", u=8, o=1, v=8)
                        .broadcast_to([128, 8, 8, 8]),
                    scalar1=1.0 / WSCALE, scalar2=None, op0=ALU.mult,
                )
                adjH[rt] = ah

            # ---- y = G*x + rep8(adj)
            # PE pieces 3,4,5: diag(G) @ x + I @ adj_bcast into PSUM, ACT copy.
            for s in (3, 4, 5, 7):
                rt, p = s >> 2, s & 3
                xin = xc[rt * 2 + (1 if p == 3 else 0)][:, (1024 * p if p < 3 else 0):][:, 0:1024]
                ys = yp.tile([128, 1024], bf16, tag="ys", name=f"y{s}")
                py = psy.tile([128, 1024], f32, tag="py", name=f"py{s}")
                for hh in range(2):                  # moving operand max 512 cols
                    sl = slice(512 * hh, 512 * (hh + 1))
                    nc.tensor.matmul(
                        out=py[:, sl], lhsT=dg_s[:, 128 * rt:128 * (rt + 1)],
                        rhs=xin[:, sl], start=True, stop=False,
                    )
                    nc.tensor.matmul(
                        out=py[:, sl], lhsT=iw_s[:, :],
                        rhs=adjH[rt][:, 128 * p + 64 * hh:128 * p + 64 * (hh + 1)]
                            .rearrange("p (hv o) -> p hv o", o=1)
                            .broadcast_to([128, 64, 8]),
                        start=False, stop=True,
                    )
                nc.scalar.copy(out=ys[:, :], in_=py[:, :])
                nc.sync.dma_start(out=yw[rt, :, 1024 * p:1024 * (p + 1)], in_=ys[:, :])

            # Vector pieces: rt0 0+1 (merged), 2; rt1 piece 6 (fused STT)
            for rt, cols, p0, cidx, xoff in ((0, 1024, 2, 0, 2048), (1, 1024, 2, 2, 2048),
                                             (0, 2048, 0, 0, 0)):
                ys = yp.tile([128, cols], bf16, tag="ys", name=f"yv{rt}{p0}")
                nc.vector.scalar_tensor_tensor(
                    out=ys[:, :].rearrange("p (hv w) -> p hv w", w=8),
                    in0=xc[cidx][:, xoff:xoff + cols]
                        .rearrange("p (hv w) -> p hv w", w=8),
                    scalar=gt_s[:, 8 * rt:8 * rt + 1],
                    in1=adjH[rt][:, 128 * p0:128 * p0 + cols // 8]
                        .rearrange("p (hv o) -> p hv o", o=1)
                        .broadcast_to([128, cols // 8, 8]),
                    op0=ALU.mult, op1=ALU.add,
                )
                nc.sync.dma_start(out=yw[rt, :, 1024 * p0:1024 * p0 + cols], in_=ys[:, :])
    nc.compile()
    return nc


_NC = None


def _get_nc():
    global _NC
    if _NC is None:
        _NC = _build_nc()
    return _NC


def _pack_weights(WL, WH):
    import ml_dtypes
    NE = 6
    # Wall[band, e, i, o, x, y]; band 0 = WL, bands 1..3 = WH[:, k-1]; e = ch*2+el
    Wall = np.empty((4, NE, C, C, 4, 4), np.float32)
    Wall[0] = WL[:NE]
    for k in range(3):
        Wall[k + 1] = WH[:NE, k]
    Wall *= 0.0625 * WSCALE            # idwt/rep8 scales + fp8 range scale
    W7 = Wall.reshape(4, 3, 2, C, C, 8, 2)            # band, ch, el, i, o, pr, par
    T = W7.transpose(0, 2, 3, 5, 1, 6, 4)             # band, el, i, pr, ch, par, o
    return np.ascontiguousarray(T.reshape(4, 128, 3072)).astype(ml_dtypes.float8_e4m3fn)


def _pack_gates(lambda_):
    lam = lambda_.reshape(B, 8).astype(np.float32)
    G = lam.sum(1)
    geff = lam[:, :6].copy()
    geff[:, 4] += lam[:, 6]
    geff[:, 5] += lam[:, 7]
    gt = np.zeros((B, 8), np.float32)
    gt[:, 0] = G
    gt[:, 1:7] = geff * 0.0625
    gt[:, 7] = -G / 64.0 * WSCALE
    return gt


def _build_in_maps(x, lambda_, WL, WH):
    import ml_dtypes
    wtp = _pack_weights(np.asarray(WL, np.float32), np.asarray(WH, np.float32))
    gtp = _pack_gates(np.asarray(lambda_, np.float32))
    xb = np.asarray(x, np.float32).astype(ml_dtypes.bfloat16)
    iw = np.ascontiguousarray(np.eye(128, dtype=np.float32)).astype(ml_dtypes.bfloat16)

    in_maps = []
    for k in range(N_CORES):
        xl = np.ascontiguousarray(xb[k * BL:(k + 1) * BL].reshape(2, 128, 4096))
        g4 = gtp[k * BL:(k + 1) * BL]                 # [4, 8], b = rt*2+bh
        gl = np.broadcast_to(
            g4.reshape(2, 2, 1, 8).transpose(1, 2, 0, 3), (2, 64, 2, 8))
        gl = np.ascontiguousarray(gl.reshape(128, 16), dtype=np.float32)
        # dg[rt] = diag over partitions (bh,c) with value G[rt*2+bh]
        Gv = g4[:, 0].reshape(2, 2)                   # [rt, bh]
        dgl = np.zeros((2, 128, 128), np.float32)
        idx = np.arange(128)
        for rt in range(2):
            dgl[rt, idx, idx] = np.repeat(Gv[rt], 64)
        in_maps.append({"xw": xl, "wt": wtp, "gt": gl,
                        "dg": dgl.astype(ml_dtypes.bfloat16),
                        "iw": iw})
    return in_maps


def kernel(x, lambda_, WL, WH):
    from concourse.bass_utils import run_bass_kernel_spmd

    nc = _get_nc()
    in_maps = _build_in_maps(x, lambda_, WL, WH)
    res = run_bass_kernel_spmd(nc, in_maps, list(range(N_CORES)))
    out = np.empty((B, C, S, S), np.float32)
    for k in range(N_CORES):
        out[k * BL:(k + 1) * BL] = (
            res.results[k]["yw"].astype(np.float32).reshape(BL, C, S, S))
    return out
